# revision 21
# baseline (speedup 1.0000x reference)
"""GNN attention layer (edge+attention MLPs, scatter-sum, node MLP, global MLP)
on 8 Trainium2 NeuronCores.

Strategy: edge-parallel sharding. Each core owns E/8 edges, sorted by
destination node (CSC layout). Gathers of x[src]/x[dst] run on-device via
indirect DMA from a bf16 node table; e rows stream in linearly (host provides
the shard pre-transposed so the feature-on-partition layout needs no on-chip
transpose). The segment-sum scatter is computed as one-hot matmuls into a
per-window PSUM accumulator (512-node windows), then a ReduceScatter combines
the 8 partial node aggregates. Node MLP runs node-sharded; the tiny global MLP
runs replicated after an 8KB AllReduce.
"""

import os
import numpy as np
import ml_dtypes

BF16 = ml_dtypes.bfloat16
F16 = np.float16

# Problem sizes (fixed by the task)
N, E, G, D, H = 50000, 800000, 16, 64, 128
C = 8                    # cores
ESH = E // C             # 100000 edges per core
P = 128                  # partitions / chunk size
W = 512                  # node window for the scatter
NPAD = 50176             # N padded: multiple of 512 and of 8
NW = NPAD // W           # 98 windows
NSH = NPAD // C          # 6272 nodes per core (node phase)
NODE_CH = NSH // P       # 49 node chunks per core

LAST_RESULT = None       # BassKernelResults of the last run (for test harness)


# ----------------------------------------------------------------------------
# Host-side preprocessing
# ----------------------------------------------------------------------------

def _prep_schedule(dst_per_core):
    """Uniform-across-cores window chunk schedule.

    Returns (chunks_w [NW], ch0_w [NW+1], NCH, EP).
    """
    cnts = np.zeros((C, NW), np.int64)
    for c, dst in enumerate(dst_per_core):
        cnts[c] = np.bincount(dst // W, minlength=NW)
    chunks_w = np.maximum(1, -(-cnts.max(axis=0) // P))   # ceil, min 1
    ch0_w = np.concatenate([[0], np.cumsum(chunks_w)]).astype(np.int64)
    NCH = int(ch0_w[-1])
    return chunks_w.astype(np.int64), ch0_w, NCH, NCH * P


def _prep_core(src, dst, e_sh, batch, chunks_w, ch0_w, NCH, EP):
    """Build one core's padded, dst-sorted device arrays."""
    order = np.argsort(dst, kind="stable")
    src_s = src[order].astype(np.int64)
    dst_s = dst[order].astype(np.int64)
    win = dst_s // W
    cnt = np.bincount(win, minlength=NW)
    starts = np.concatenate([[0], np.cumsum(cnt)])
    # padded slot of each sorted edge
    rank = np.arange(len(dst_s)) - starts[win]
    slot = ch0_w[win] * P + rank                     # [ESH]
    pp = slot % P
    ch = slot // P

    srcidx = np.zeros((P, NCH), np.int32)
    dstidx = np.zeros((P, NCH), np.int32)
    dstl = np.full((P, NCH), -1.0, np.float32)
    bsrcc = np.full((P, NCH), -1.0, np.float32)
    bsrcr = np.full((16, EP), -1.0, BF16)
    e_padT = np.zeros((D, EP), np.float32)

    bsrc = batch[src_s].astype(np.float32)
    srcidx[pp, ch] = src_s
    dstidx[pp, ch] = dst_s
    dstl[pp, ch] = (dst_s - win * W).astype(np.float32)
    bsrcc[pp, ch] = bsrc
    bsrcr[:, slot] = bsrc.astype(BF16)[None, :]
    e_padT[:, slot] = e_sh[order].T.astype(np.float32)

    return dict(order=order, slot=slot, srcidx=srcidx, dstidx=dstidx,
                dstl=dstl, bsrcc=bsrcc, bsrcr=bsrcr, e_padT=e_padT)


def _prep_weights(u, We1, be1, We2, be2, Wa1, ba1, Wa2, ba2,
                  Wn1, bn1, Wn2, bn2, Wg1, bg1, Wg2, bg2):
    u = u.astype(np.float64)
    f32 = np.float32

    def b16(a):
        return np.ascontiguousarray(a).astype(BF16)

    out = dict(
        w1xs=b16(We1[0:64]), w1xd=b16(We1[64:128]),              # [64,128]
        w1eu=b16(np.concatenate([We1[128:192],
                                 (u @ We1[192:256].astype(np.float64))], 0)),
        waxs=b16(Wa1[0:64]), waxd=b16(Wa1[64:128]),
        waeu=b16(np.concatenate([Wa1[128:192],
                                 (u @ Wa1[192:256].astype(np.float64))], 0)),
        we2=b16(We2), wa2=b16(Wa2),
        be1c=be1.reshape(H, 1).astype(f32),
        ba1c=ba1.reshape(H, 1).astype(f32),
        be2c=be2.reshape(D, 1).astype(f32),
        be2r=np.tile(be2.astype(f32), (P, 1)),                   # [128,64]
        ba2r=np.tile(ba2.astype(f32), (P, 1)),
        wnxa=b16(Wn1[0:128]),
        un16=b16(u @ Wn1[128:192].astype(np.float64)),           # [16,128]
        wn2=b16(Wn2),
        bn1c=bn1.reshape(H, 1).astype(f32),
        bn2r=np.tile(bn2.astype(f32), (P, 1)),
        wgun=b16(Wg1[0:128]),
        wge=b16(Wg1[128:192]),
        wg2=b16(Wg2),
        bg1c=bg1.reshape(H, 1).astype(f32),
        bg2r=np.tile(bg2.astype(f32), (G, 1)),                   # [16,64]
        ut16=b16(u.T),                                           # [64,16]
        io512=np.tile(np.arange(W, dtype=F16), (P, 1)),          # [128,512]
        io16f=np.tile(np.arange(16, dtype=F16), (P, 1)),         # [128,16]
        io16b=np.tile(np.arange(16).astype(BF16), (P, 1)),       # [128,16]
        io16c=np.arange(16, dtype=np.float32).reshape(16, 1),    # [16,1]
        io16cx=_io16cx(),                                        # [128,1]
    )
    return out


def _io16cx():
    a = np.zeros((P, 1), np.float32)
    a[64:80, 0] = np.arange(16, dtype=np.float32)
    return a


# ----------------------------------------------------------------------------
# Device program
# ----------------------------------------------------------------------------

def _build_program(chunks_w, ch0_w, NCH, EP):
    import concourse.bass as bass
    import concourse.bacc as bacc
    import concourse.mybir as mybir
    import concourse.tile as tile
    from concourse.masks import make_identity

    dt = mybir.dt
    AO = mybir.AluOpType
    AF = mybir.ActivationFunctionType

    nc = bacc.Bacc("TRN2", target_bir_lowering=False, debug=False,
                   num_devices=C)

    # inputs -----------------------------------------------------------------
    def inp(name, shape, dty):
        return nc.dram_tensor(name, list(shape), dty, kind="ExternalInput")

    x_t = inp("xtab", [NPAD, D], dt.bfloat16)
    xsh_t = inp("xshard", [NSH, D], dt.bfloat16)
    ept_t = inp("epadT", [D, EP], dt.float32)
    si_t = inp("srcidx", [P, NCH], dt.int32)
    di_t = inp("dstidx", [P, NCH], dt.int32)
    dl_t = inp("dstl", [P, NCH], dt.float32)
    bc_t = inp("bsrcc", [P, NCH], dt.float32)
    br_t = inp("bsrcr", [16, EP], dt.bfloat16)
    bnc_t = inp("bnodec", [P, NODE_CH], dt.float32)
    bnr_t = inp("bnoder", [16, NSH], dt.bfloat16)

    wspec = dict(
        w1xs=([64, 128], dt.bfloat16), w1xd=([64, 128], dt.bfloat16),
        w1eu=([80, 128], dt.bfloat16),
        waxs=([64, 128], dt.bfloat16), waxd=([64, 128], dt.bfloat16),
        waeu=([80, 128], dt.bfloat16),
        we2=([128, 64], dt.bfloat16), wa2=([128, 64], dt.bfloat16),
        be1c=([128, 1], dt.float32), ba1c=([128, 1], dt.float32),
        be2c=([64, 1], dt.float32),
        be2r=([128, 64], dt.float32), ba2r=([128, 64], dt.float32),
        wnxa=([128, 128], dt.bfloat16), un16=([16, 128], dt.bfloat16),
        wn2=([128, 64], dt.bfloat16),
        bn1c=([128, 1], dt.float32), bn2r=([128, 64], dt.float32),
        wgun=([128, 128], dt.bfloat16), wge=([64, 128], dt.bfloat16),
        wg2=([128, 64], dt.bfloat16),
        bg1c=([128, 1], dt.float32), bg2r=([16, 64], dt.float32),
        ut16=([64, 16], dt.bfloat16),
        io512=([128, W], dt.float16), io16f=([128, 16], dt.float16),
        io16b=([128, 16], dt.bfloat16), io16c=([16, 1], dt.float32),
        io16cx=([128, 1], dt.float32),
    )
    w_t = {k: inp(k, s, d) for k, (s, d) in wspec.items()}

    # outputs ----------------------------------------------------------------
    enewT_o = nc.dram_tensor("enewT", [D, EP], dt.float32,
                             kind="ExternalOutput")
    xnew_o = nc.dram_tensor("xnew", [NSH, D], dt.float32,
                            kind="ExternalOutput")
    unew_o = nc.dram_tensor("unew", [G, D], dt.float32, kind="ExternalOutput")

    # internal DRAM ----------------------------------------------------------
    aggd = nc.dram_tensor("aggbuf", [NPAD, D], dt.float32, kind="Internal")
    dbg_agg = (nc.dram_tensor("dbg_agg", [NPAD, D], dt.float32,
                              kind="ExternalOutput")
               if os.environ.get("GNN_DEBUG_AGG") == "1" else None)
    rs_o = nc.dram_tensor("rsout", [NSH, D], dt.float32, kind="Internal")
    ar2i = nc.dram_tensor("ar2i", [128, 16], dt.float32, kind="Internal")
    ar2o = nc.dram_tensor("ar2o", [128, 16], dt.float32, kind="Internal",
                          addr_space="Shared")

    with tile.TileContext(nc) as tc:
        with (
            tc.tile_pool(name="const", bufs=1) as cst,
            tc.tile_pool(name="win", bufs=4) as wp,
            tc.tile_pool(name="chk", bufs=8) as cp,
            tc.tile_pool(name="ptr", bufs=2, space="PSUM") as ptr,
            tc.tile_pool(name="ph", bufs=2, space="PSUM") as phh,
            tc.tile_pool(name="ps", bufs=2, space="PSUM") as pss,
            tc.tile_pool(name="pagg", bufs=2, space="PSUM") as pagg,
        ):
            # constants to SBUF
            wt = {}
            for k, (s, d) in wspec.items():
                wt[k] = cst.tile(s, d, tag=f"w_{k}", name=f"w_{k}")
                nc.sync.dma_start(wt[k][:], w_t[k].ap())
            ident = cst.tile([128, 128], dt.bfloat16, tag="ident")
            make_identity(nc, ident[:])
            ident16 = cst.tile([128, 128], dt.float16, tag="ident16")
            make_identity(nc, ident16[:])
            eagg_acc = cst.tile([64, 16], dt.float32, tag="eagg")
            nc.vector.memset(eagg_acc[:], 0.0)
            bnc_sb = cst.tile([P, NODE_CH], dt.float32, tag="bnc")
            nc.sync.dma_start(bnc_sb[:], bnc_t.ap())
            bnr_sb = cst.tile([16, NSH], dt.bfloat16, tag="bnr")
            nc.sync.dma_start(bnr_sb[:], bnr_t.ap())

            maxcw = int(chunks_w.max())

            # ---------------- edge phase ----------------
            for w in range(NW):
                cw = int(chunks_w[w])
                ch0 = int(ch0_w[w])
                j0 = ch0 * P
                jn = cw * P

                sidx = wp.tile([P, cw], dt.int32, tag="sidx")
                dstl = wp.tile([P, cw], dt.float32, tag="dstl")
                bsc = wp.tile([P, cw], dt.float32, tag="bsc")
                bsr = wp.tile([80, jn], dt.bfloat16, tag="bsr")
                didx = wp.tile([P, cw], dt.int32, tag="didx")
                nc.scalar.dma_start(sidx[:], si_t.ap()[:, ch0:ch0 + cw])
                nc.scalar.dma_start(didx[:], di_t.ap()[:, ch0:ch0 + cw])
                nc.scalar.dma_start(dstl[:], dl_t.ap()[:, ch0:ch0 + cw])
                nc.scalar.dma_start(bsc[:], bc_t.ap()[:, ch0:ch0 + cw])
                nc.scalar.dma_start(bsr[64:80, :], br_t.ap()[:, j0:j0 + jn])

                ein = wp.tile([64, jn], dt.float32, tag="ein")
                nc.sync.dma_start(ein[:], ept_t.ap()[:, j0:j0 + jn])

                xsT_sl = wp.tile([64, jn], dt.bfloat16, tag="xsTs")
                xdT_sl = wp.tile([64, jn], dt.bfloat16, tag="xdTs")
                h1r_sl = wp.tile([128, jn], dt.bfloat16, tag="h1rs")
                h2r_sl = wp.tile([128, jn], dt.bfloat16, tag="h2rs")
                eT_sl = wp.tile([80, jn], dt.bfloat16, tag="eT")
                e2T_sl = wp.tile([80, jn], dt.bfloat16, tag="e2T")
                enT_f = wp.tile([64, jn], dt.float32, tag="enT")
                nc.vector.tensor_copy(eT_sl[0:64, :], ein[:])

                ps_agg = pagg.tile([128, 288], dt.float32, tag="agg")

                for k in range(cw):
                    cs = slice(k * P, k * P + P)
                    # xs/xd: per-chunk indirect gather + PE transpose to slabs
                    xsn = cp.tile([128, 64], dt.bfloat16, tag="xsn")
                    nc.gpsimd.indirect_dma_start(
                        out=xsn[:], out_offset=None, in_=x_t.ap(),
                        in_offset=bass.IndirectOffsetOnAxis(
                            ap=sidx[:, k:k + 1], axis=0))
                    pxs = ptr.tile([64, 128], dt.bfloat16, tag="tr")
                    nc.tensor.transpose(pxs[:], xsn[:], ident[:])
                    nc.vector.tensor_copy(xsT_sl[:, cs], pxs[:])
                    xdn = cp.tile([128, 64], dt.bfloat16, tag="xdn")
                    nc.gpsimd.indirect_dma_start(
                        out=xdn[:], out_offset=None, in_=x_t.ap(),
                        in_offset=bass.IndirectOffsetOnAxis(
                            ap=didx[:, k:k + 1], axis=0))
                    pxd = ptr.tile([64, 128], dt.bfloat16, tag="tr")
                    nc.tensor.transpose(pxd[:], xdn[:], ident[:])
                    nc.vector.tensor_copy(xdT_sl[:, cs], pxd[:])

                # u one-hot slab-wide (graph-on-partition) into both rhs slabs
                nc.vector.tensor_scalar(
                    eT_sl[64:80, :], bsr[64:80, :], wt["io16cx"][64:80, :],
                    None, AO.is_equal)
                nc.vector.tensor_scalar(
                    e2T_sl[64:80, :], bsr[64:80, :], wt["io16cx"][64:80, :],
                    None, AO.is_equal)

                # MLP hiddens + e_newT in 512-wide blocks
                nb = (jn + 511) // 512
                for b in range(nb):
                    bs = slice(b * 512, min((b + 1) * 512, jn))
                    bw = bs.stop - bs.start
                    ph = phh.tile([128, 512], dt.float32, tag="h")
                    nc.tensor.matmul(ph[:, :bw], wt["w1xs"][:], xsT_sl[:, bs],
                                     start=True, stop=False)
                    nc.tensor.matmul(ph[:, :bw], wt["w1xd"][:], xdT_sl[:, bs],
                                     start=False, stop=False)
                    nc.tensor.matmul(ph[:, :bw], wt["w1eu"][:], eT_sl[:, bs],
                                     start=False, stop=True)
                    nc.vector.tensor_scalar(h1r_sl[:, bs], ph[:, :bw],
                                            wt["be1c"][:], 0.0, AO.add, AO.max)
                    ps1 = pss.tile([128, 512], dt.float32, tag="s")
                    nc.tensor.matmul(ps1[0:64, :bw], wt["we2"][:],
                                     h1r_sl[:, bs], start=True, stop=True)
                    nc.vector.tensor_scalar(enT_f[:, bs], ps1[0:64, :bw],
                                            wt["be2c"][:], None, AO.add)
                    nc.vector.tensor_copy(e2T_sl[0:64, bs], enT_f[:, bs])
                    ph2 = phh.tile([128, 512], dt.float32, tag="h")
                    nc.tensor.matmul(ph2[:, :bw], wt["waxs"][:], xsT_sl[:, bs],
                                     start=True, stop=False)
                    nc.tensor.matmul(ph2[:, :bw], wt["waxd"][:], xdT_sl[:, bs],
                                     start=False, stop=False)
                    nc.tensor.matmul(ph2[:, :bw], wt["waeu"][:], e2T_sl[:, bs],
                                     start=False, stop=True)
                    nc.vector.tensor_scalar(h2r_sl[:, bs], ph2[:, :bw],
                                            wt["ba1c"][:], 0.0, AO.add, AO.max)

                for k in range(cw):
                    cs = slice(k * P, k * P + P)
                    first, last = (k == 0), (k == cw - 1)
                    oh = cp.tile([128, W], dt.float16, tag="oh")
                    nc.vector.tensor_scalar(oh[:], wt["io512"][:],
                                            dstl[:, k:k + 1], None,
                                            AO.is_equal)

                    # natural-layout e_new and a (second layers, swapped form)
                    pe = pss.tile([128, 128], dt.float32, tag="s")
                    nc.tensor.matmul(pe[:, 0:64], h1r_sl[:, cs], wt["we2"][:],
                                     start=True, stop=True)
                    en16 = cp.tile([128, 64], dt.float16, tag="en16")
                    nc.vector.tensor_tensor(
                        out=en16[:], in0=pe[:, 0:64], in1=wt["be2r"][:],
                        op=AO.add)
                    pa = pss.tile([128, 128], dt.float32, tag="s")
                    nc.tensor.matmul(pa[:, 0:64], h2r_sl[:, cs], wt["wa2"][:],
                                     start=True, stop=True)
                    af = cp.tile([128, 64], dt.float32, tag="af")
                    nc.vector.tensor_tensor(
                        out=af[:], in0=pa[:, 0:64], in1=wt["ba2r"][:],
                        op=AO.add)
                    a16 = cp.tile([128, 64], dt.float16, tag="a16")
                    nc.scalar.activation(a16[:], af[:], AF.Sigmoid)
                    m16 = cp.tile([128, 64], dt.float16, tag="m16")
                    nc.vector.tensor_tensor(out=m16[:], in0=en16[:],
                                            in1=a16[:], op=AO.mult)

                    # one-hot for edge_agg
                    uh = cp.tile([128, 16], dt.float16, tag="uh")
                    nc.vector.tensor_scalar(uh[:], wt["io16f"][:],
                                            bsc[:, k:k + 1], None,
                                            AO.is_equal)

                    for s in range(4):
                        nc.tensor.matmul(
                            ps_agg[:, 64 * s:64 * s + 64],
                            oh[:, 128 * s:128 * s + 128], m16[:],
                            start=(first and s == 0), stop=last)
                    nc.tensor.matmul(ps_agg[0:64, 256:272], en16[:], uh[:],
                                     start=False, stop=last)

                # window drain
                agg_sb = wp.tile([128, 256], dt.float32, tag="aggsb")
                nc.vector.tensor_copy(agg_sb[:], ps_agg[:, 0:256])
                out_ap = aggd.ap()[W * w:W * w + W, :].rearrange(
                    "(s p) d -> p s d", p=P)
                nc.sync.dma_start(
                    out_ap, agg_sb[:].rearrange("p (s d) -> p s d", d=D))
                if dbg_agg is not None:
                    dbg_ap = dbg_agg.ap()[W * w:W * w + W, :].rearrange(
                        "(s p) d -> p s d", p=P)
                    nc.sync.dma_start(
                        dbg_ap, agg_sb[:].rearrange("p (s d) -> p s d", d=D))
                nc.vector.tensor_tensor(out=eagg_acc[:], in0=eagg_acc[:],
                                        in1=ps_agg[0:64, 256:272], op=AO.add)

                nc.sync.dma_start(enewT_o.ap()[:, j0:j0 + jn], enT_f[:])

            # ---------------- reduce-scatter of node aggregate -------------
            nc.gpsimd.collective_compute(
                "ReduceScatter", AO.add,
                replica_groups=[list(range(C))],
                ins=[aggd.ap().opt()], outs=[rs_o.ap().opt()])

            # ---------------- node phase ----------------
            pnagg = pagg.tile([128, 16], dt.float32, tag="agg")
            for k in range(NODE_CH):
                rs = slice(k * P, k * P + P)
                xnd = wp.tile([128, 64], dt.bfloat16, tag="xnd")
                nc.sync.dma_start(xnd[:], xsh_t.ap()[rs, :])
                agf = wp.tile([128, 64], dt.float32, tag="agf")
                nc.sync.dma_start(agf[:], rs_o.ap()[rs, :])
                agb = cp.tile([128, 64], dt.bfloat16, tag="agb")
                nc.vector.tensor_copy(agb[:], agf[:])

                nT = cp.tile([128, 128], dt.bfloat16, tag="nT")
                pn = ptr.tile([128, 128], dt.bfloat16, tag="tr")
                nc.tensor.transpose(pn[0:64, :], xnd[:], ident[:])
                nc.tensor.transpose(pn[64:128, :], agb[:], ident[:])
                nc.vector.tensor_copy(nT[:], pn[:])

                uhT = cp.tile([16, 128], dt.bfloat16, tag="uhT")
                nc.vector.tensor_scalar(uhT[:], bnr_sb[:, rs],
                                        wt["io16c"][:], None, AO.is_equal)

                ph = phh.tile([128, 128], dt.float32, tag="h")
                nc.tensor.matmul(ph[:], wt["wnxa"][:], nT[:],
                                 start=True, stop=False)
                nc.tensor.matmul(ph[:], wt["un16"][:], uhT[:],
                                 start=False, stop=True)
                hnr = cp.tile([128, 128], dt.bfloat16, tag="h1r")
                nc.vector.tensor_scalar(hnr[:], ph[:], wt["bn1c"][:], 0.0,
                                        AO.add, AO.max)

                px = pss.tile([128, 128], dt.float32, tag="s")
                nc.tensor.matmul(px[:, 0:64], hnr[:], wt["wn2"][:],
                                 start=True, stop=True)
                xnf = cp.tile([128, 64], dt.float32, tag="xnf")
                nc.vector.tensor_tensor(out=xnf[:], in0=px[:, 0:64],
                                        in1=wt["bn2r"][:], op=AO.add)
                nc.sync.dma_start(xnew_o.ap()[rs, :], xnf[:])

                xnb = cp.tile([128, 64], dt.bfloat16, tag="xnb")
                nc.vector.tensor_copy(xnb[:], xnf[:])
                uhn = cp.tile([128, 16], dt.bfloat16, tag="uhn")
                nc.vector.tensor_scalar(uhn[:], wt["io16b"][:],
                                        bnc_sb[:, k:k + 1], None,
                                        AO.is_equal)
                nc.tensor.matmul(pnagg[0:64, :], xnb[:], uhn[:],
                                 start=(k == 0), stop=(k == NODE_CH - 1))

            # ---------------- global phase ----------------
            nag_sb = cp.tile([64, 16], dt.float32, tag="nagsb")
            nc.vector.tensor_copy(nag_sb[:], pnagg[0:64, :])
            nc.sync.dma_start(ar2i.ap()[0:64, :], eagg_acc[:])
            nc.sync.dma_start(ar2i.ap()[64:128, :], nag_sb[:])
            nc.gpsimd.collective_compute(
                "AllReduce", AO.add,
                replica_groups=[list(range(C))],
                ins=[ar2i.ap().opt()], outs=[ar2o.ap().opt()])

            gag = cp.tile([128, 16], dt.float32, tag="gag")
            nc.sync.dma_start(gag[:], ar2o.ap())
            gab = cp.tile([128, 16], dt.bfloat16, tag="gab")
            nc.vector.tensor_copy(gab[:], gag[:])
            rhs1 = cp.tile([128, 16], dt.bfloat16, tag="rhs1")
            nc.vector.tensor_copy(rhs1[0:64, :], wt["ut16"][:])
            nc.vector.tensor_copy(rhs1[64:128, :], gab[64:128, :])

            phu = phh.tile([128, 16], dt.float32, tag="h")
            nc.tensor.matmul(phu[:], wt["wgun"][:], rhs1[:],
                             start=True, stop=False)
            nc.tensor.matmul(phu[:], wt["wge"][:], gab[0:64, :],
                             start=False, stop=True)
            hur = cp.tile([128, 16], dt.bfloat16, tag="hur")
            nc.vector.tensor_scalar(hur[:], phu[:], wt["bg1c"][:], 0.0,
                                    AO.add, AO.max)
            pu = pss.tile([128, 64], dt.float32, tag="s")
            nc.tensor.matmul(pu[0:16, :], hur[:], wt["wg2"][:],
                             start=True, stop=True)
            unf = cp.tile([16, 64], dt.float32, tag="unf")
            nc.vector.tensor_tensor(out=unf[:], in0=pu[0:16, :],
                                    in1=wt["bg2r"][:], op=AO.add)
            nc.sync.dma_start(unew_o.ap(), unf[:])

    nc.compile()
    return nc


def kernel(x, edge_index, e, u, batch,
           We1, be1, We2, be2, Wa1, ba1, Wa2, ba2,
           Wn1, bn1, Wn2, bn2, Wg1, bg1, Wg2, bg2):
    from concourse.bass_utils import run_bass_kernel_spmd

    x = np.asarray(x); edge_index = np.asarray(edge_index)
    e = np.asarray(e); u = np.asarray(u); batch = np.asarray(batch)
    args = [np.asarray(a) for a in (We1, be1, We2, be2, Wa1, ba1, Wa2, ba2,
                                    Wn1, bn1, Wn2, bn2, Wg1, bg1, Wg2, bg2)]

    src_all, dst_all = edge_index[0].astype(np.int64), edge_index[1].astype(np.int64)
    dst_per_core = [dst_all[c * ESH:(c + 1) * ESH] for c in range(C)]
    chunks_w, ch0_w, NCH, EP = _prep_schedule(dst_per_core)

    cores = []
    for c in range(C):
        sl = slice(c * ESH, (c + 1) * ESH)
        cores.append(_prep_core(src_all[sl], dst_per_core[c], e[sl], batch,
                                chunks_w, ch0_w, NCH, EP))

    wts = _prep_weights(u, *args)

    x_b16 = np.zeros((NPAD, D), BF16)
    x_b16[:N] = x.astype(BF16)
    batch_pad = np.full(NPAD, -1.0, np.float32)
    batch_pad[:N] = batch.astype(np.float32)

    in_maps = []
    for c in range(C):
        cc = cores[c]
        nsl = slice(c * NSH, (c + 1) * NSH)
        bnodec = np.ascontiguousarray(
            batch_pad[nsl].reshape(NODE_CH, P).T).astype(np.float32)
        bnoder = np.tile(batch_pad[nsl].astype(BF16), (16, 1))
        m = dict(xtab=x_b16, xshard=x_b16[nsl].copy(),
                 epadT=cc["e_padT"], srcidx=cc["srcidx"], dstidx=cc["dstidx"],
                 dstl=cc["dstl"], bsrcc=cc["bsrcc"], bsrcr=cc["bsrcr"],
                 bnodec=bnodec, bnoder=bnoder)
        m.update(wts)
        in_maps.append(m)

    nc = _build_program(chunks_w, ch0_w, NCH, EP)

    trace = os.environ.get("GNN_TRACE") == "1"
    tmpdir = os.environ.get("GNN_TRACE_DIR") or None
    res = run_bass_kernel_spmd(nc, in_maps, core_ids=list(range(C)),
                               trace=trace, tmpdir=tmpdir)
    global LAST_RESULT
    LAST_RESULT = res

    # gather / unshard
    e_new = np.empty((E, D), np.float32)
    for c in range(C):
        cc = cores[c]
        sh = np.empty((ESH, D), np.float32)
        sh[cc["order"]] = res.results[c]["enewT"].T[cc["slot"]]
        e_new[c * ESH:(c + 1) * ESH] = sh

    x_new = np.empty((N, D), np.float32)
    for c in range(C):
        lo = c * NSH
        real = min(NSH, N - lo)
        if real > 0:
            x_new[lo:lo + real] = res.results[c]["xnew"][:real]

    u_new = res.results[0]["unew"].astype(np.float32)
    return (x_new, e_new, u_new)


# revision 22
# speedup vs baseline: 1.3056x; 1.3056x over previous
"""GNN attention layer (edge+attention MLPs, scatter-sum, node MLP, global MLP)
on 8 Trainium2 NeuronCores.

Strategy: edge-parallel sharding. Each core owns E/8 edges, sorted by
destination node (CSC layout). Gathers of x[src]/x[dst] run on-device via
indirect DMA from a bf16 node table; e rows stream in linearly (host provides
the shard pre-transposed so the feature-on-partition layout needs no on-chip
transpose). The segment-sum scatter is computed as one-hot matmuls into a
per-window PSUM accumulator (512-node windows), then a ReduceScatter combines
the 8 partial node aggregates. Node MLP runs node-sharded; the tiny global MLP
runs replicated after an 8KB AllReduce.
"""

import os
import numpy as np
import ml_dtypes

BF16 = ml_dtypes.bfloat16
F16 = np.float16

# Problem sizes (fixed by the task)
N, E, G, D, H = 50000, 800000, 16, 64, 128
C = 8                    # cores
ESH = E // C             # 100000 edges per core
P = 128                  # partitions / chunk size
W = 512                  # node window for the scatter
NPAD = 50176             # N padded: multiple of 512 and of 8
NW = NPAD // W           # 98 windows
NSH = NPAD // C          # 6272 nodes per core (node phase)
NODE_CH = NSH // P       # 49 node chunks per core

LAST_RESULT = None       # BassKernelResults of the last run (for test harness)


# ----------------------------------------------------------------------------
# Host-side preprocessing
# ----------------------------------------------------------------------------

def _prep_schedule(dst_per_core):
    """Uniform-across-cores window chunk schedule.

    Returns (chunks_w [NW], ch0_w [NW+1], NCH, EP).
    """
    cnts = np.zeros((C, NW), np.int64)
    for c, dst in enumerate(dst_per_core):
        cnts[c] = np.bincount(dst // W, minlength=NW)
    chunks_w = np.maximum(1, -(-cnts.max(axis=0) // P))   # ceil, min 1
    ch0_w = np.concatenate([[0], np.cumsum(chunks_w)]).astype(np.int64)
    NCH = int(ch0_w[-1])
    return chunks_w.astype(np.int64), ch0_w, NCH, NCH * P


def _prep_core(src, dst, e_sh, batch, chunks_w, ch0_w, NCH, EP):
    """Build one core's padded, dst-sorted device arrays."""
    order = np.argsort(dst, kind="stable")
    src_s = src[order].astype(np.int64)
    dst_s = dst[order].astype(np.int64)
    win = dst_s // W
    cnt = np.bincount(win, minlength=NW)
    starts = np.concatenate([[0], np.cumsum(cnt)])
    # padded slot of each sorted edge
    rank = np.arange(len(dst_s)) - starts[win]
    slot = ch0_w[win] * P + rank                     # [ESH]
    pp = slot % P
    ch = slot // P

    srcidx = np.zeros((P, NCH), np.int32)
    dstidx = np.zeros((P, NCH), np.int32)
    dstl = np.full((P, NCH), -1.0, np.float32)
    bsrcc = np.full((P, NCH), -1.0, np.float32)
    bsrcr = np.full((16, EP), -1.0, BF16)
    e_padT = np.zeros((D, EP), np.float32)

    bsrc = batch[src_s].astype(np.float32)
    srcidx[pp, ch] = src_s
    dstidx[pp, ch] = dst_s
    dstl[pp, ch] = (dst_s - win * W).astype(np.float32)
    bsrcc[pp, ch] = bsrc
    bsrcr[:, slot] = bsrc.astype(BF16)[None, :]
    e_padT[:, slot] = e_sh[order].T.astype(np.float32)

    return dict(order=order, slot=slot, srcidx=srcidx, dstidx=dstidx,
                dstl=dstl, bsrcc=bsrcc, bsrcr=bsrcr, e_padT=e_padT)


def _prep_weights(u, We1, be1, We2, be2, Wa1, ba1, Wa2, ba2,
                  Wn1, bn1, Wn2, bn2, Wg1, bg1, Wg2, bg2):
    u = u.astype(np.float64)
    f32 = np.float32

    def b16(a):
        return np.ascontiguousarray(a).astype(BF16)

    out = dict(
        w1xs=b16(We1[0:64]), w1xd=b16(We1[64:128]),              # [64,128]
        w1eu=b16(np.concatenate([We1[128:192],
                                 (u @ We1[192:256].astype(np.float64))], 0)),
        waxs=b16(Wa1[0:64]), waxd=b16(Wa1[64:128]),
        waeu=b16(np.concatenate([Wa1[128:192],
                                 (u @ Wa1[192:256].astype(np.float64))], 0)),
        we2=b16(We2), wa2=b16(Wa2),
        be1c=be1.reshape(H, 1).astype(f32),
        ba1c=ba1.reshape(H, 1).astype(f32),
        be2c=be2.reshape(D, 1).astype(f32),
        be2r=np.tile(be2.astype(f32), (P, 1)),                   # [128,64]
        ba2r=np.tile(ba2.astype(f32), (P, 1)),
        wnxa=b16(Wn1[0:128]),
        un16=b16(u @ Wn1[128:192].astype(np.float64)),           # [16,128]
        wn2=b16(Wn2),
        bn1c=bn1.reshape(H, 1).astype(f32),
        bn2r=np.tile(bn2.astype(f32), (P, 1)),
        wgun=b16(Wg1[0:128]),
        wge=b16(Wg1[128:192]),
        wg2=b16(Wg2),
        bg1c=bg1.reshape(H, 1).astype(f32),
        bg2r=np.tile(bg2.astype(f32), (G, 1)),                   # [16,64]
        ut16=b16(u.T),                                           # [64,16]
        io512=np.tile(np.arange(W, dtype=F16), (P, 1)),          # [128,512]
        io16f=np.tile(np.arange(16, dtype=F16), (P, 1)),         # [128,16]
        io16b=np.tile(np.arange(16).astype(BF16), (P, 1)),       # [128,16]
        io16c=np.arange(16, dtype=np.float32).reshape(16, 1),    # [16,1]
        io16cx=_io16cx(),                                        # [128,1]
    )
    return out


def _io16cx():
    a = np.zeros((P, 1), np.float32)
    a[64:80, 0] = np.arange(16, dtype=np.float32)
    return a


# ----------------------------------------------------------------------------
# Device program
# ----------------------------------------------------------------------------

def _build_program(chunks_w, ch0_w, NCH, EP):
    import concourse.bass as bass
    import concourse.bacc as bacc
    import concourse.mybir as mybir
    import concourse.tile as tile
    from concourse.masks import make_identity

    dt = mybir.dt
    AO = mybir.AluOpType
    AF = mybir.ActivationFunctionType

    nc = bacc.Bacc("TRN2", target_bir_lowering=False, debug=False,
                   num_devices=C)

    # inputs -----------------------------------------------------------------
    def inp(name, shape, dty):
        return nc.dram_tensor(name, list(shape), dty, kind="ExternalInput")

    x_t = inp("xtab", [NPAD, D], dt.bfloat16)
    xsh_t = inp("xshard", [NSH, D], dt.bfloat16)
    ept_t = inp("epadT", [D, EP], dt.float32)
    si_t = inp("srcidx", [P, NCH], dt.int32)
    di_t = inp("dstidx", [P, NCH], dt.int32)
    dl_t = inp("dstl", [P, NCH], dt.float32)
    bc_t = inp("bsrcc", [P, NCH], dt.float32)
    br_t = inp("bsrcr", [16, EP], dt.bfloat16)
    bnc_t = inp("bnodec", [P, NODE_CH], dt.float32)
    bnr_t = inp("bnoder", [16, NSH], dt.bfloat16)

    wspec = dict(
        w1xs=([64, 128], dt.bfloat16), w1xd=([64, 128], dt.bfloat16),
        w1eu=([80, 128], dt.bfloat16),
        waxs=([64, 128], dt.bfloat16), waxd=([64, 128], dt.bfloat16),
        waeu=([80, 128], dt.bfloat16),
        we2=([128, 64], dt.bfloat16), wa2=([128, 64], dt.bfloat16),
        be1c=([128, 1], dt.float32), ba1c=([128, 1], dt.float32),
        be2c=([64, 1], dt.float32),
        be2r=([128, 64], dt.float32), ba2r=([128, 64], dt.float32),
        wnxa=([128, 128], dt.bfloat16), un16=([16, 128], dt.bfloat16),
        wn2=([128, 64], dt.bfloat16),
        bn1c=([128, 1], dt.float32), bn2r=([128, 64], dt.float32),
        wgun=([128, 128], dt.bfloat16), wge=([64, 128], dt.bfloat16),
        wg2=([128, 64], dt.bfloat16),
        bg1c=([128, 1], dt.float32), bg2r=([16, 64], dt.float32),
        ut16=([64, 16], dt.bfloat16),
        io512=([128, W], dt.float16), io16f=([128, 16], dt.float16),
        io16b=([128, 16], dt.bfloat16), io16c=([16, 1], dt.float32),
        io16cx=([128, 1], dt.float32),
    )
    w_t = {k: inp(k, s, d) for k, (s, d) in wspec.items()}

    # outputs ----------------------------------------------------------------
    enewT_o = nc.dram_tensor("enewT", [D, EP], dt.float32,
                             kind="ExternalOutput")
    xnew_o = nc.dram_tensor("xnew", [NSH, D], dt.float32,
                            kind="ExternalOutput")
    unew_o = nc.dram_tensor("unew", [G, D], dt.float32, kind="ExternalOutput")

    # internal DRAM ----------------------------------------------------------
    aggd = nc.dram_tensor("aggbuf", [NPAD, D], dt.float32, kind="Internal")
    dbg_agg = (nc.dram_tensor("dbg_agg", [NPAD, D], dt.float32,
                              kind="ExternalOutput")
               if os.environ.get("GNN_DEBUG_AGG") == "1" else None)
    rs_o = nc.dram_tensor("rsout", [NSH, D], dt.float32, kind="Internal")
    ar2i = nc.dram_tensor("ar2i", [128, 16], dt.float32, kind="Internal")
    ar2o = nc.dram_tensor("ar2o", [128, 16], dt.float32, kind="Internal",
                          addr_space="Shared")

    with tile.TileContext(nc) as tc:
        with (
            tc.tile_pool(name="const", bufs=1) as cst,
            tc.tile_pool(name="win", bufs=4) as wp,
            tc.tile_pool(name="chk", bufs=5) as cp,
            tc.tile_pool(name="ptr", bufs=2, space="PSUM") as ptr,
            tc.tile_pool(name="ph", bufs=2, space="PSUM") as phh,
            tc.tile_pool(name="ps", bufs=2, space="PSUM") as pss,
            tc.tile_pool(name="pagg", bufs=2, space="PSUM") as pagg,
        ):
            # constants to SBUF
            wt = {}
            for k, (s, d) in wspec.items():
                wt[k] = cst.tile(s, d, tag=f"w_{k}", name=f"w_{k}")
                nc.sync.dma_start(wt[k][:], w_t[k].ap())
            ident = cst.tile([128, 128], dt.bfloat16, tag="ident")
            make_identity(nc, ident[:])
            ident16 = cst.tile([128, 128], dt.float16, tag="ident16")
            make_identity(nc, ident16[:])
            eagg_acc = cst.tile([64, 16], dt.float32, tag="eagg")
            nc.vector.memset(eagg_acc[:], 0.0)
            bnc_sb = cst.tile([P, NODE_CH], dt.float32, tag="bnc")
            nc.sync.dma_start(bnc_sb[:], bnc_t.ap())
            bnr_sb = cst.tile([16, NSH], dt.bfloat16, tag="bnr")
            nc.sync.dma_start(bnr_sb[:], bnr_t.ap())

            maxcw = int(chunks_w.max())

            # ---------------- edge phase ----------------
            for w in range(NW):
                cw = int(chunks_w[w])
                ch0 = int(ch0_w[w])
                j0 = ch0 * P
                jn = cw * P

                sidx = wp.tile([P, cw], dt.int32, tag="sidx")
                dstl = wp.tile([P, cw], dt.float32, tag="dstl")
                bsc = wp.tile([P, cw], dt.float32, tag="bsc")
                bsr = wp.tile([80, jn], dt.bfloat16, tag="bsr")
                didx = wp.tile([P, cw], dt.int32, tag="didx")
                nc.scalar.dma_start(sidx[:], si_t.ap()[:, ch0:ch0 + cw])
                nc.scalar.dma_start(didx[:], di_t.ap()[:, ch0:ch0 + cw])
                nc.scalar.dma_start(dstl[:], dl_t.ap()[:, ch0:ch0 + cw])
                nc.scalar.dma_start(bsc[:], bc_t.ap()[:, ch0:ch0 + cw])
                nc.scalar.dma_start(bsr[64:80, :], br_t.ap()[:, j0:j0 + jn])

                ein = wp.tile([64, jn], dt.float32, tag="ein")
                nc.sync.dma_start(ein[:], ept_t.ap()[:, j0:j0 + jn])

                xsT_sl = wp.tile([64, jn], dt.bfloat16, tag="xsTs")
                xdT_sl = wp.tile([64, jn], dt.bfloat16, tag="xdTs")
                h1r_sl = wp.tile([128, jn], dt.bfloat16, tag="h1rs")
                h2r_sl = wp.tile([128, jn], dt.bfloat16, tag="h2rs")
                eT_sl = wp.tile([80, jn], dt.bfloat16, tag="eT")
                e2T_sl = wp.tile([80, jn], dt.bfloat16, tag="e2T")
                enT_f = wp.tile([64, jn], dt.float32, tag="enT")
                nc.vector.tensor_copy(eT_sl[0:64, :], ein[:])

                ps_agg = pagg.tile([128, 288], dt.float32, tag="agg")

                for k in range(cw):
                    cs = slice(k * P, k * P + P)
                    # xs/xd: per-chunk indirect gather + PE transpose to slabs
                    xsn = cp.tile([128, 64], dt.bfloat16, tag="xsn")
                    nc.gpsimd.indirect_dma_start(
                        out=xsn[:], out_offset=None, in_=x_t.ap(),
                        in_offset=bass.IndirectOffsetOnAxis(
                            ap=sidx[:, k:k + 1], axis=0))
                    pxs = ptr.tile([64, 128], dt.bfloat16, tag="tr")
                    nc.tensor.transpose(pxs[:], xsn[:], ident[:])
                    nc.vector.tensor_copy(xsT_sl[:, cs], pxs[:])
                    xdn = cp.tile([128, 64], dt.bfloat16, tag="xdn")
                    nc.gpsimd.indirect_dma_start(
                        out=xdn[:], out_offset=None, in_=x_t.ap(),
                        in_offset=bass.IndirectOffsetOnAxis(
                            ap=didx[:, k:k + 1], axis=0))
                    pxd = ptr.tile([64, 128], dt.bfloat16, tag="tr")
                    nc.tensor.transpose(pxd[:], xdn[:], ident[:])
                    nc.vector.tensor_copy(xdT_sl[:, cs], pxd[:])

                # u one-hot slab-wide (graph-on-partition) into both rhs slabs
                nc.vector.tensor_scalar(
                    eT_sl[64:80, :], bsr[64:80, :], wt["io16cx"][64:80, :],
                    None, AO.is_equal)
                nc.vector.tensor_scalar(
                    e2T_sl[64:80, :], bsr[64:80, :], wt["io16cx"][64:80, :],
                    None, AO.is_equal)

                # MLP hiddens + e_newT in 512-wide blocks
                nb = (jn + 511) // 512
                for b in range(nb):
                    bs = slice(b * 512, min((b + 1) * 512, jn))
                    bw = bs.stop - bs.start
                    ph = phh.tile([128, 512], dt.float32, tag="h")
                    nc.tensor.matmul(ph[:, :bw], wt["w1xs"][:], xsT_sl[:, bs],
                                     start=True, stop=False)
                    nc.tensor.matmul(ph[:, :bw], wt["w1xd"][:], xdT_sl[:, bs],
                                     start=False, stop=False)
                    nc.tensor.matmul(ph[:, :bw], wt["w1eu"][:], eT_sl[:, bs],
                                     start=False, stop=True)
                    nc.vector.tensor_scalar(h1r_sl[:, bs], ph[:, :bw],
                                            wt["be1c"][:], 0.0, AO.add, AO.max)
                    ps1 = pss.tile([128, 512], dt.float32, tag="s")
                    nc.tensor.matmul(ps1[0:64, :bw], wt["we2"][:],
                                     h1r_sl[:, bs], start=True, stop=True)
                    nc.vector.tensor_scalar(enT_f[:, bs], ps1[0:64, :bw],
                                            wt["be2c"][:], None, AO.add)
                    nc.vector.tensor_copy(e2T_sl[0:64, bs], enT_f[:, bs])
                    ph2 = phh.tile([128, 512], dt.float32, tag="h")
                    nc.tensor.matmul(ph2[:, :bw], wt["waxs"][:], xsT_sl[:, bs],
                                     start=True, stop=False)
                    nc.tensor.matmul(ph2[:, :bw], wt["waxd"][:], xdT_sl[:, bs],
                                     start=False, stop=False)
                    nc.tensor.matmul(ph2[:, :bw], wt["waeu"][:], e2T_sl[:, bs],
                                     start=False, stop=True)
                    nc.vector.tensor_scalar(h2r_sl[:, bs], ph2[:, :bw],
                                            wt["ba1c"][:], 0.0, AO.add, AO.max)

                for k in range(cw):
                    cs = slice(k * P, k * P + P)
                    first, last = (k == 0), (k == cw - 1)
                    oh = cp.tile([128, W], dt.float16, tag="oh")
                    nc.vector.tensor_scalar(oh[:], wt["io512"][:],
                                            dstl[:, k:k + 1], None,
                                            AO.is_equal)

                    # natural-layout e_new and a (second layers, swapped form)
                    pe = pss.tile([128, 128], dt.float32, tag="s")
                    nc.tensor.matmul(pe[:, 0:64], h1r_sl[:, cs], wt["we2"][:],
                                     start=True, stop=True)
                    en16 = cp.tile([128, 64], dt.float16, tag="en16")
                    nc.vector.tensor_tensor(
                        out=en16[:], in0=pe[:, 0:64], in1=wt["be2r"][:],
                        op=AO.add)
                    pa = pss.tile([128, 128], dt.float32, tag="s")
                    nc.tensor.matmul(pa[:, 0:64], h2r_sl[:, cs], wt["wa2"][:],
                                     start=True, stop=True)
                    af = cp.tile([128, 64], dt.float32, tag="af")
                    nc.vector.tensor_tensor(
                        out=af[:], in0=pa[:, 0:64], in1=wt["ba2r"][:],
                        op=AO.add)
                    a16 = cp.tile([128, 64], dt.float16, tag="a16")
                    nc.scalar.activation(a16[:], af[:], AF.Sigmoid)
                    m16 = cp.tile([128, 64], dt.float16, tag="m16")
                    nc.vector.tensor_tensor(out=m16[:], in0=en16[:],
                                            in1=a16[:], op=AO.mult)

                    # one-hot for edge_agg
                    uh = cp.tile([128, 16], dt.float16, tag="uh")
                    nc.vector.tensor_scalar(uh[:], wt["io16f"][:],
                                            bsc[:, k:k + 1], None,
                                            AO.is_equal)

                    for s in range(4):
                        nc.tensor.matmul(
                            ps_agg[:, 64 * s:64 * s + 64],
                            oh[:, 128 * s:128 * s + 128], m16[:],
                            start=(first and s == 0), stop=last)
                    nc.tensor.matmul(ps_agg[0:64, 256:272], en16[:], uh[:],
                                     start=False, stop=last)

                # window drain
                agg_sb = wp.tile([128, 256], dt.float32, tag="aggsb")
                nc.vector.tensor_copy(agg_sb[:], ps_agg[:, 0:256])
                out_ap = aggd.ap()[W * w:W * w + W, :].rearrange(
                    "(s p) d -> p s d", p=P)
                nc.sync.dma_start(
                    out_ap, agg_sb[:].rearrange("p (s d) -> p s d", d=D))
                if dbg_agg is not None:
                    dbg_ap = dbg_agg.ap()[W * w:W * w + W, :].rearrange(
                        "(s p) d -> p s d", p=P)
                    nc.sync.dma_start(
                        dbg_ap, agg_sb[:].rearrange("p (s d) -> p s d", d=D))
                nc.vector.tensor_tensor(out=eagg_acc[:], in0=eagg_acc[:],
                                        in1=ps_agg[0:64, 256:272], op=AO.add)

                nc.sync.dma_start(enewT_o.ap()[:, j0:j0 + jn], enT_f[:])

            # ---------------- reduce-scatter of node aggregate -------------
            nc.gpsimd.collective_compute(
                "ReduceScatter", AO.add,
                replica_groups=[list(range(C))],
                ins=[aggd.ap().opt()], outs=[rs_o.ap().opt()])

            # ---------------- node phase ----------------
            pnagg = pagg.tile([128, 16], dt.float32, tag="agg")
            for k in range(NODE_CH):
                rs = slice(k * P, k * P + P)
                xnd = wp.tile([128, 64], dt.bfloat16, tag="xnd")
                nc.sync.dma_start(xnd[:], xsh_t.ap()[rs, :])
                agf = wp.tile([128, 64], dt.float32, tag="agf")
                nc.sync.dma_start(agf[:], rs_o.ap()[rs, :])
                agb = cp.tile([128, 64], dt.bfloat16, tag="agb")
                nc.vector.tensor_copy(agb[:], agf[:])

                nT = cp.tile([128, 128], dt.bfloat16, tag="nT")
                pn = ptr.tile([128, 128], dt.bfloat16, tag="tr")
                nc.tensor.transpose(pn[0:64, :], xnd[:], ident[:])
                nc.tensor.transpose(pn[64:128, :], agb[:], ident[:])
                nc.vector.tensor_copy(nT[:], pn[:])

                uhT = cp.tile([16, 128], dt.bfloat16, tag="uhT")
                nc.vector.tensor_scalar(uhT[:], bnr_sb[:, rs],
                                        wt["io16c"][:], None, AO.is_equal)

                ph = phh.tile([128, 128], dt.float32, tag="h")
                nc.tensor.matmul(ph[:], wt["wnxa"][:], nT[:],
                                 start=True, stop=False)
                nc.tensor.matmul(ph[:], wt["un16"][:], uhT[:],
                                 start=False, stop=True)
                hnr = cp.tile([128, 128], dt.bfloat16, tag="h1r")
                nc.vector.tensor_scalar(hnr[:], ph[:], wt["bn1c"][:], 0.0,
                                        AO.add, AO.max)

                px = pss.tile([128, 128], dt.float32, tag="s")
                nc.tensor.matmul(px[:, 0:64], hnr[:], wt["wn2"][:],
                                 start=True, stop=True)
                xnf = cp.tile([128, 64], dt.float32, tag="xnf")
                nc.vector.tensor_tensor(out=xnf[:], in0=px[:, 0:64],
                                        in1=wt["bn2r"][:], op=AO.add)
                nc.sync.dma_start(xnew_o.ap()[rs, :], xnf[:])

                xnb = cp.tile([128, 64], dt.bfloat16, tag="xnb")
                nc.vector.tensor_copy(xnb[:], xnf[:])
                uhn = cp.tile([128, 16], dt.bfloat16, tag="uhn")
                nc.vector.tensor_scalar(uhn[:], wt["io16b"][:],
                                        bnc_sb[:, k:k + 1], None,
                                        AO.is_equal)
                nc.tensor.matmul(pnagg[0:64, :], xnb[:], uhn[:],
                                 start=(k == 0), stop=(k == NODE_CH - 1))

            # ---------------- global phase ----------------
            nag_sb = cp.tile([64, 16], dt.float32, tag="nagsb")
            nc.vector.tensor_copy(nag_sb[:], pnagg[0:64, :])
            nc.sync.dma_start(ar2i.ap()[0:64, :], eagg_acc[:])
            nc.sync.dma_start(ar2i.ap()[64:128, :], nag_sb[:])
            nc.gpsimd.collective_compute(
                "AllReduce", AO.add,
                replica_groups=[list(range(C))],
                ins=[ar2i.ap().opt()], outs=[ar2o.ap().opt()])

            gag = cp.tile([128, 16], dt.float32, tag="gag")
            nc.sync.dma_start(gag[:], ar2o.ap())
            gab = cp.tile([128, 16], dt.bfloat16, tag="gab")
            nc.vector.tensor_copy(gab[:], gag[:])
            rhs1 = cp.tile([128, 16], dt.bfloat16, tag="rhs1")
            nc.vector.tensor_copy(rhs1[0:64, :], wt["ut16"][:])
            nc.vector.tensor_copy(rhs1[64:128, :], gab[64:128, :])

            phu = phh.tile([128, 16], dt.float32, tag="h")
            nc.tensor.matmul(phu[:], wt["wgun"][:], rhs1[:],
                             start=True, stop=False)
            nc.tensor.matmul(phu[:], wt["wge"][:], gab[0:64, :],
                             start=False, stop=True)
            hur = cp.tile([128, 16], dt.bfloat16, tag="hur")
            nc.vector.tensor_scalar(hur[:], phu[:], wt["bg1c"][:], 0.0,
                                    AO.add, AO.max)
            pu = pss.tile([128, 64], dt.float32, tag="s")
            nc.tensor.matmul(pu[0:16, :], hur[:], wt["wg2"][:],
                             start=True, stop=True)
            unf = cp.tile([16, 64], dt.float32, tag="unf")
            nc.vector.tensor_tensor(out=unf[:], in0=pu[0:16, :],
                                    in1=wt["bg2r"][:], op=AO.add)
            nc.sync.dma_start(unew_o.ap(), unf[:])

    nc.compile()
    return nc


def kernel(x, edge_index, e, u, batch,
           We1, be1, We2, be2, Wa1, ba1, Wa2, ba2,
           Wn1, bn1, Wn2, bn2, Wg1, bg1, Wg2, bg2):
    from concourse.bass_utils import run_bass_kernel_spmd

    x = np.asarray(x); edge_index = np.asarray(edge_index)
    e = np.asarray(e); u = np.asarray(u); batch = np.asarray(batch)
    args = [np.asarray(a) for a in (We1, be1, We2, be2, Wa1, ba1, Wa2, ba2,
                                    Wn1, bn1, Wn2, bn2, Wg1, bg1, Wg2, bg2)]

    src_all, dst_all = edge_index[0].astype(np.int64), edge_index[1].astype(np.int64)
    dst_per_core = [dst_all[c * ESH:(c + 1) * ESH] for c in range(C)]
    chunks_w, ch0_w, NCH, EP = _prep_schedule(dst_per_core)

    cores = []
    for c in range(C):
        sl = slice(c * ESH, (c + 1) * ESH)
        cores.append(_prep_core(src_all[sl], dst_per_core[c], e[sl], batch,
                                chunks_w, ch0_w, NCH, EP))

    wts = _prep_weights(u, *args)

    x_b16 = np.zeros((NPAD, D), BF16)
    x_b16[:N] = x.astype(BF16)
    batch_pad = np.full(NPAD, -1.0, np.float32)
    batch_pad[:N] = batch.astype(np.float32)

    in_maps = []
    for c in range(C):
        cc = cores[c]
        nsl = slice(c * NSH, (c + 1) * NSH)
        bnodec = np.ascontiguousarray(
            batch_pad[nsl].reshape(NODE_CH, P).T).astype(np.float32)
        bnoder = np.tile(batch_pad[nsl].astype(BF16), (16, 1))
        m = dict(xtab=x_b16, xshard=x_b16[nsl].copy(),
                 epadT=cc["e_padT"], srcidx=cc["srcidx"], dstidx=cc["dstidx"],
                 dstl=cc["dstl"], bsrcc=cc["bsrcc"], bsrcr=cc["bsrcr"],
                 bnodec=bnodec, bnoder=bnoder)
        m.update(wts)
        in_maps.append(m)

    nc = _build_program(chunks_w, ch0_w, NCH, EP)

    trace = os.environ.get("GNN_TRACE") == "1"
    tmpdir = os.environ.get("GNN_TRACE_DIR") or None
    res = run_bass_kernel_spmd(nc, in_maps, core_ids=list(range(C)),
                               trace=trace, tmpdir=tmpdir)
    global LAST_RESULT
    LAST_RESULT = res

    # gather / unshard
    e_new = np.empty((E, D), np.float32)
    for c in range(C):
        cc = cores[c]
        sh = np.empty((ESH, D), np.float32)
        sh[cc["order"]] = res.results[c]["enewT"].T[cc["slot"]]
        e_new[c * ESH:(c + 1) * ESH] = sh

    x_new = np.empty((N, D), np.float32)
    for c in range(C):
        lo = c * NSH
        real = min(NSH, N - lo)
        if real > 0:
            x_new[lo:lo + real] = res.results[c]["xnew"][:real]

    u_new = res.results[0]["unew"].astype(np.float32)
    return (x_new, e_new, u_new)
